# revision 13
# baseline (speedup 1.0000x reference)
"""Trainium2 Bass kernel: grouped MoE expert MLP (nn_ExpertGroup).

Strategy: expert parallelism across 8 NeuronCores. Tokens are sorted by
expert; core e runs expert e's two GEMMs:
    h = relu(x_e @ w_up[e].T) ** 2      (bf16, like the CUDA reference)
    y = h @ w_down[e].T
The host does the (free) token scatter/gather, the bf16 casts, and packs
every device-side DMA chunk into a fully contiguous DRAM block, so each
dma_start is 128 descriptors of 1-8KB at full transfer rate instead of
1024 strided 256B-1KB rows.

Timing-critical discipline (measured on hardware): the PE's DVFS boost
clock (2.4 GHz vs 2.0 GHz base) is earned during an early activity
window and is forfeited FOR THE WHOLE RUN if the PE idles more than
~2us. So (1) warm-up dummies bridge the preamble to the first operand
arrival with no gap, and (2) the input DMA schedule must keep every
w_up j-tile's completion semaphore ahead of the PE's consumption. Input
DMAs are split across the two HWDGE queues (Sync + Scalar/Activation,
each ~150-230 GB/s, ~350 GB/s aggregate) so the first GEMM1 chunk's
operands (x tokens 0:256 + w_up j0) land ~2us earlier than a single
queue could, and the j-tile stream is supplied from both queues
alternately at ~0.75us cadence vs the PE's 0.88us/j-tile demand.

Device layout (per core, cap = padded local token count, default 1024):
    xT_sb  [128, 8*cap]      bf16  x_e.T packed per (chunk, d, tok)
    wuT_sb [128, 16, 8, 128] bf16  w_up[e].T packed per (j, d, col)
    wdT_sb [128, 16, 1024]   bf16  w_down[e].T packed per (j4, col)
    GEMM1: psum[j,t] = sum_d wuT[j,d].T @ xT[d,t]   (h in [H, T] layout)
           token chunks [256, 256, 512] so the first chunk needs only
           512KB of x before the stream can start
    DVE:   relu -> bf16, square -> hsq SBUF [128, 16, cap]
    GEMM2: psum[t,i] = sum_j hsq[j,t].T @ wdT[j,i]  (y in [T, D] layout)
    DVE:   cast fp32 psum -> bf16 -> DMA to packed y [128, 8, 1024]
    The final GEMM2 group is split 256+256 so the serial drain after the
    last matmul (cast + output DMA + completion) moves half the bytes.

Built on bacc.Bacc (not raw Bass): Bacc.compile() legalizes semaphore
waits to the TRN2 limit of one wait per instruction (moving matmul waits
onto ldweights and splitting the rest into EventSemaphore instructions).
Raw Bass BIR fails walrus codegen with "Too many sync wait commands".
"""

import numpy as np
import ml_dtypes

import concourse.bass as bass
import concourse.mybir as mybir
import concourse.tile as tile
from concourse import bacc
from concourse.bass_utils import run_bass_kernel_spmd

T, D, H, E = 8192, 1024, 2048, 8
P = 128
N_CORES = 8
FD = 512           # GEMM2 matmul moving free dim (one PSUM bank of fp32)
C_CHUNKS = [(0, 256), (256, 256), (512, 512)]  # GEMM1 token chunks
N_WARM = 58        # PE warm-up dummies (bridge preamble -> first operands)
LAST_SPLIT = 256   # final GEMM2 group split size (drain shortening)


def _ensure_axon_ntff_hook():
    """The container's `antenv` stub lacks `axon_hooks`; if BASS_TRACE=1 is
    set, run_bass_kernel_spmd would crash importing it. Recreate the tiny
    registry and register the ctypes NTFF hook so tracing works (and never
    let this best-effort setup break the kernel)."""
    try:
        import antenv.axon_hooks  # noqa: F401
        return
    except ImportError:
        pass
    try:
        import sys
        import types

        import antenv
        from trn_agent_boot.trn_boot import _ntff_profile_via_ctypes

        mod = types.ModuleType("antenv.axon_hooks")
        mod._hook = _ntff_profile_via_ctypes("/opt/axon/libaxon_pjrt.so")
        mod.set_axon_ntff_profile_hook = lambda h: setattr(mod, "_hook", h)
        mod.get_axon_ntff_profile_hook = lambda: mod._hook
        sys.modules["antenv.axon_hooks"] = mod
        antenv.axon_hooks = mod
    except Exception:
        pass


_ensure_axon_ntff_hook()

_PROGRAM_CACHE: dict[int, "bass.Bass"] = {}
LAST_RESULT = None  # BassKernelResults of the most recent run (for harness use)


def _build_program(cap: int) -> "bass.Bass":
    n_d = D // P       # 8  contraction tiles of GEMM1
    n_j = H // P       # 16 H partition tiles
    n_t = cap // P     # token partition tiles (GEMM2 output)
    assert sum(l for _, l in C_CHUNKS) == cap
    bf16 = mybir.dt.bfloat16
    f32 = mybir.dt.float32

    nc = bacc.Bacc("TRN2", debug=False, num_devices=N_CORES)
    # Packed inputs: every tensor is consumed by exactly one dma_start and
    # is contiguous in DRAM in the order that DMA writes SBUF.
    xch = [
        nc.dram_tensor(f"x{c}", [P, n_d * l], bf16, kind="ExternalInput")
        for c, (_, l) in enumerate(C_CHUNKS)
    ]
    wuch = [
        nc.dram_tensor(f"wu{j}", [P, n_d * P], bf16, kind="ExternalInput")
        for j in range(n_j)
    ]
    wdch = [
        nc.dram_tensor(f"wd{c}", [P, 4 * D], bf16, kind="ExternalInput")
        for c in range(n_j // 4)
    ]
    y = nc.dram_tensor("y", [P, n_t * D], bf16, kind="ExternalOutput")

    with tile.TileContext(nc) as tc:
        with (
            tc.tile_pool(name="big", bufs=1) as big,
            tc.tile_pool(name="outp", bufs=4) as outp,
            tc.tile_pool(name="actp", bufs=4) as actp,
            tc.tile_pool(name="psum", bufs=7, space="PSUM") as psum,
            tc.tile_pool(name="warmp", bufs=1, space="PSUM") as warmp,
        ):
            xT_sb = big.tile([P, n_d * cap], bf16)
            wuT_sb = big.tile([P, n_j, n_d, P], bf16)
            wdT_sb = big.tile([P, n_j, D], bf16)
            hsq_sb = big.tile([P, n_j, cap], bf16)

            # PE warm-up: dummy matmuls with no DMA dependencies run while
            # the first input DMAs land, ending right at the measured
            # first-operand semaphore time. Any >2us PE idle here loses the
            # DVFS boost clock for the whole run (-20% on every matmul).
            warm = big.tile([P, P], bf16)
            nc.vector.memset(warm[:], 0.0)
            wps = warmp.tile([P, P], f32, tag="warm")
            for _ in range(N_WARM):
                nc.tensor.matmul(wps, warm[:], warm[:], start=True, stop=True)

            # --- input DMAs, split across the two HWDGE queues ---
            # Transfers on one queue are processed in issue order at
            # ~150-230 GB/s, so each queue is an independent supply lane.
            # Scalar lane: x chunk 0 first (the start-gating transfer),
            # then even w_up j-tiles, then x chunks 1-2, then w_down tail.
            # Sync lane: w_up j0 (the other start-gating transfer), then
            # odd w_up j-tiles, then the first two w_down chunks.
            # Scalar lane: x chunk 0 first (the start-gating transfer),
            # then even w_up j-tiles, then x chunks 1-2, then the w_down
            # tail (needed only ~55us in, at GEMM2). Sync lane: w_up j0
            # (the other gating transfer), odd j-tiles, first two w_down.
            q_sync = [("x", 0)] + [("wu", j) for j in range(1, n_j, 2)]
            q_sync += [("wd", 0), ("wd", 1)]
            q_scalar = [("wu", 0)] + [("wu", j) for j in range(2, n_j, 2)]
            q_scalar += [("x", 1), ("x", 2), ("wd", 2), ("wd", 3)]

            def issue(eng, kind, i):
                if kind == "x":
                    s, l = C_CHUNKS[i]
                    eng.dma_start(
                        out=xT_sb[:, n_d * s:n_d * (s + l)], in_=xch[i][:]
                    )
                elif kind == "wu":
                    eng.dma_start(out=wuT_sb[:, i], in_=wuch[i][:])
                else:
                    eng.dma_start(
                        out=wdT_sb[:, i * 4:(i + 1) * 4, :], in_=wdch[i][:]
                    )

            for kind, i in q_scalar:
                issue(nc.scalar, kind, i)
            for kind, i in q_sync:
                issue(nc.sync, kind, i)

            # --- GEMM1 + relu^2: hsq[j, t] ---
            for ci, (s, l) in enumerate(C_CHUNKS):
                xbase = n_d * s
                for j in range(n_j):
                    ps = psum.tile([P, FD], f32, tag="ps")
                    for d in range(n_d):
                        nc.tensor.matmul(
                            ps[:, 0:l],
                            wuT_sb[:, j, d],
                            xT_sb[:, xbase + d * l:xbase + (d + 1) * l],
                            start=(d == 0),
                            stop=(d == n_d - 1),
                        )
                    hr = actp.tile([P, FD], bf16, tag="hr")
                    nc.vector.tensor_relu(out=hr[:, 0:l], in_=ps[:, 0:l])
                    nc.vector.tensor_mul(
                        out=hsq_sb[:, j, s:s + l], in0=hr[:, 0:l], in1=hr[:, 0:l]
                    )

            # --- GEMM2: y[t, i] = sum_j hsq[j, t].T @ wdT[j, i] ---
            groups = []
            for t in range(n_t):
                for ic in range(D // FD):
                    lo, hi = ic * FD, (ic + 1) * FD
                    if t == n_t - 1 and hi == D and LAST_SPLIT:
                        groups.append((t, lo, hi - LAST_SPLIT))
                        groups.append((t, hi - LAST_SPLIT, hi))
                    else:
                        groups.append((t, lo, hi))
            for gi, (t, lo, hi) in enumerate(groups):
                w = hi - lo
                ps = psum.tile([P, FD], f32, tag="ps")
                for j in range(n_j):
                    nc.tensor.matmul(
                        ps[:, 0:w],
                        hsq_sb[:, j, t * P:(t + 1) * P],
                        wdT_sb[:, j, lo:hi],
                        start=(j == 0),
                        stop=(j == n_j - 1),
                    )
                # The very last group drains serially after the final
                # matmul: cast + DMA it in two halves so the first DMA
                # trigger overlaps the second cast and the final transfer
                # is half the bytes.
                halves = [(0, w, nc.sync)] if gi < len(groups) - 1 else [
                    (0, w // 2, nc.sync), (w // 2, w, nc.scalar)
                ]
                for a, b, eng in halves:
                    yt = outp.tile([P, FD], bf16, tag="yt")
                    nc.vector.tensor_copy(out=yt[:, 0:b - a], in_=ps[:, a:b])
                    eng.dma_start(
                        out=y[:, t * D + lo + a:t * D + lo + b],
                        in_=yt[:, 0:b - a],
                    )

    nc.compile()
    return nc


def _get_program(cap: int) -> "bass.Bass":
    nc = _PROGRAM_CACHE.get(cap)
    if nc is None:
        nc = _build_program(cap)
        _PROGRAM_CACHE[cap] = nc
    return nc


CAP = 1024  # tokens per core per round (the uniform T/E split = one round)


def kernel(x, num_tokens_per_expert, w_up, w_down, _trace=False):
    global LAST_RESULT
    bf = ml_dtypes.bfloat16
    x = np.asarray(x)
    counts = np.asarray(num_tokens_per_expert).astype(np.int64)
    w_up = np.asarray(w_up)
    w_down = np.asarray(w_down)
    n_tok = x.shape[0]
    assert counts.shape == (E,) and int(counts.sum()) == n_tok
    offsets = np.zeros(E, dtype=np.int64)
    offsets[1:] = np.cumsum(counts)[:-1]

    nc = _get_program(CAP)
    n_d, n_j, n_t = D // P, H // P, CAP // P

    # Work list: split each expert's contiguous token segment into slots of
    # <= CAP tokens; process 8 slots per SPMD round. The uniform T/E = 1024
    # split is exactly one round of 8 slots.
    slots = []
    for e in range(E):
        cnt, off = int(counts[e]), int(offsets[e])
        for s in range(0, cnt, CAP):
            slots.append((e, off + s, min(CAP, cnt - s)))

    weight_cache = {}

    def expert_weights(e):
        if e not in weight_cache:
            wuT = np.ascontiguousarray(w_up[e].astype(bf).T)    # [D, H]
            wdT = np.ascontiguousarray(w_down[e].astype(bf).T)  # [H, D]
            wu3 = wuT.reshape(n_d, P, H)
            m = {
                f"wu{j}": np.ascontiguousarray(
                    wu3[:, :, j * P:(j + 1) * P].transpose(1, 0, 2)
                ).reshape(P, n_d * P)
                for j in range(n_j)
            }
            wd3 = wdT.reshape(n_j, P, D)
            for c in range(n_j // 4):
                m[f"wd{c}"] = np.ascontiguousarray(
                    wd3[c * 4:(c + 1) * 4].transpose(1, 0, 2)
                ).reshape(P, 4 * D)
            weight_cache[e] = m
        return weight_cache[e]

    out = np.zeros((n_tok, D), dtype=x.dtype)
    zero_map = None
    for r0 in range(0, len(slots), N_CORES):
        round_slots = slots[r0:r0 + N_CORES]
        in_maps = []
        for e, off, cnt in round_slots:
            xs = np.zeros((CAP, D), dtype=bf)
            xs[:cnt] = x[off:off + cnt].astype(bf)
            xT = np.ascontiguousarray(xs.T)  # [D, CAP]
            xT3 = xT.reshape(n_d, P, CAP)
            im = dict(expert_weights(e))
            for c, (s, l) in enumerate(C_CHUNKS):
                im[f"x{c}"] = np.ascontiguousarray(
                    xT3[:, :, s:s + l].transpose(1, 0, 2)
                ).reshape(P, n_d * l)
            in_maps.append(im)
        while len(in_maps) < N_CORES:  # idle cores in the last round
            if zero_map is None:
                zero_map = {
                    f"x{c}": np.zeros((P, n_d * l), dtype=bf)
                    for c, (_, l) in enumerate(C_CHUNKS)
                }
                zero_map.update({
                    f"wu{j}": np.zeros((P, n_d * P), dtype=bf)
                    for j in range(n_j)
                })
                zero_map.update({
                    f"wd{c}": np.zeros((P, 4 * D), dtype=bf)
                    for c in range(n_j // 4)
                })
            in_maps.append(zero_map)

        res = run_bass_kernel_spmd(
            nc, in_maps, core_ids=list(range(N_CORES)), trace=_trace
        )
        LAST_RESULT = res
        for i, (e, off, cnt) in enumerate(round_slots):
            yp = res.results[i]["y"].reshape(P, n_t, D).transpose(1, 0, 2)
            out[off:off + cnt] = yp.reshape(CAP, D)[:cnt].astype(x.dtype)
    return out


# revision 15
# speedup vs baseline: 1.0132x; 1.0132x over previous
"""Trainium2 Bass kernel: grouped MoE expert MLP (nn_ExpertGroup).

Strategy: expert parallelism across 8 NeuronCores. Tokens are sorted by
expert; core e runs expert e's two GEMMs:
    h = relu(x_e @ w_up[e].T) ** 2      (bf16, like the CUDA reference)
    y = h @ w_down[e].T
The host does the (free) token scatter/gather, the bf16 casts, and packs
every device-side DMA chunk into a fully contiguous DRAM block, so each
dma_start is 128 descriptors of 1-8KB at full transfer rate instead of
1024 strided 256B-1KB rows.

Timing-critical discipline (measured on hardware): the PE's DVFS boost
clock (2.4 GHz vs 2.0 GHz base) is earned during an early activity
window and is forfeited FOR THE WHOLE RUN if the PE idles more than
~2us. So (1) warm-up dummies bridge the preamble to the first operand
arrival with no gap, and (2) the input DMA schedule must keep every
w_up j-tile's completion semaphore ahead of the PE's consumption. Input
DMAs are split across the two HWDGE queues (Sync + Scalar/Activation,
each ~150-230 GB/s, ~350 GB/s aggregate) so the first GEMM1 chunk's
operands (x tokens 0:256 + w_up j0) land ~2us earlier than a single
queue could, and the j-tile stream is supplied from both queues
alternately at ~0.75us cadence vs the PE's 0.88us/j-tile demand.

Device layout (per core, cap = padded local token count, default 1024):
    xT_sb  [128, 8*cap]      bf16  x_e.T packed per (chunk, d, tok)
    wuT_sb [128, 16, 8, 128] bf16  w_up[e].T packed per (j, d, col)
    wdT_sb [128, 16, 1024]   bf16  w_down[e].T packed per (j4, col)
    GEMM1: psum[j,t] = sum_d wuT[j,d].T @ xT[d,t]   (h in [H, T] layout)
           token chunks [256, 256, 512] so the first chunk needs only
           512KB of x before the stream can start
    DVE:   relu -> bf16, square -> hsq SBUF [128, 16, cap]
    GEMM2: psum[t,i] = sum_j hsq[j,t].T @ wdT[j,i]  (y in [T, D] layout)
    DVE:   cast fp32 psum -> bf16 -> DMA to packed y [128, 8, 1024]
    The final GEMM2 group is split 256+256 so the serial drain after the
    last matmul (cast + output DMA + completion) moves half the bytes.

Built on bacc.Bacc (not raw Bass): Bacc.compile() legalizes semaphore
waits to the TRN2 limit of one wait per instruction (moving matmul waits
onto ldweights and splitting the rest into EventSemaphore instructions).
Raw Bass BIR fails walrus codegen with "Too many sync wait commands".
"""

import numpy as np
import ml_dtypes

import concourse.bass as bass
import concourse.mybir as mybir
import concourse.tile as tile
from concourse import bacc
from concourse.bass_utils import run_bass_kernel_spmd

T, D, H, E = 8192, 1024, 2048, 8
P = 128
N_CORES = 8
FD = 512           # GEMM2 matmul moving free dim (one PSUM bank of fp32)
C_CHUNKS = [(0, 256), (256, 256), (512, 512)]  # GEMM1 token chunks
N_WARM = 54        # PE warm-up dummies (bridge preamble -> first operands)
LAST_SPLIT = 256   # final GEMM2 group split size (drain shortening)


def _ensure_axon_ntff_hook():
    """The container's `antenv` stub lacks `axon_hooks`; if BASS_TRACE=1 is
    set, run_bass_kernel_spmd would crash importing it. Recreate the tiny
    registry and register the ctypes NTFF hook so tracing works (and never
    let this best-effort setup break the kernel)."""
    try:
        import antenv.axon_hooks  # noqa: F401
        return
    except ImportError:
        pass
    try:
        import sys
        import types

        import antenv
        from trn_agent_boot.trn_boot import _ntff_profile_via_ctypes

        mod = types.ModuleType("antenv.axon_hooks")
        mod._hook = _ntff_profile_via_ctypes("/opt/axon/libaxon_pjrt.so")
        mod.set_axon_ntff_profile_hook = lambda h: setattr(mod, "_hook", h)
        mod.get_axon_ntff_profile_hook = lambda: mod._hook
        sys.modules["antenv.axon_hooks"] = mod
        antenv.axon_hooks = mod
    except Exception:
        pass


_ensure_axon_ntff_hook()

_PROGRAM_CACHE: dict[int, "bass.Bass"] = {}
LAST_RESULT = None  # BassKernelResults of the most recent run (for harness use)


def _build_program(cap: int) -> "bass.Bass":
    n_d = D // P       # 8  contraction tiles of GEMM1
    n_j = H // P       # 16 H partition tiles
    n_t = cap // P     # token partition tiles (GEMM2 output)
    assert sum(l for _, l in C_CHUNKS) == cap
    bf16 = mybir.dt.bfloat16
    f32 = mybir.dt.float32

    nc = bacc.Bacc("TRN2", debug=False, num_devices=N_CORES)
    # Packed inputs: every tensor is consumed by exactly one dma_start and
    # is contiguous in DRAM in the order that DMA writes SBUF.
    xch = [
        nc.dram_tensor(f"x{c}", [P, n_d * l], bf16, kind="ExternalInput")
        for c, (_, l) in enumerate(C_CHUNKS)
    ]
    wuch = [
        nc.dram_tensor(f"wu{j}", [P, n_d * P], bf16, kind="ExternalInput")
        for j in range(n_j)
    ]
    wdch = [
        nc.dram_tensor(f"wd{c}", [P, 4 * D], bf16, kind="ExternalInput")
        for c in range(n_j // 4)
    ]
    y = nc.dram_tensor("y", [P, n_t * D], bf16, kind="ExternalOutput")

    with tile.TileContext(nc) as tc:
        with (
            tc.tile_pool(name="big", bufs=1) as big,
            tc.tile_pool(name="outp", bufs=4) as outp,
            tc.tile_pool(name="actp", bufs=4) as actp,
            tc.tile_pool(name="psum", bufs=7, space="PSUM") as psum,
            tc.tile_pool(name="warmp", bufs=1, space="PSUM") as warmp,
        ):
            xT_sb = big.tile([P, n_d * cap], bf16)
            wuT_sb = big.tile([P, n_j, n_d, P], bf16)
            wdT_sb = big.tile([P, n_j, D], bf16)
            hsq_sb = big.tile([P, n_j, cap], bf16)

            # PE warm-up: dummy matmuls with no DMA dependencies run while
            # the first input DMAs land, ending right at the measured
            # first-operand semaphore time. Any >2us PE idle here loses the
            # DVFS boost clock for the whole run (-20% on every matmul).
            warm = big.tile([P, P], bf16)
            nc.vector.memset(warm[:], 0.0)
            wps = warmp.tile([P, P], f32, tag="warm")
            for _ in range(N_WARM):
                nc.tensor.matmul(wps, warm[:], warm[:], start=True, stop=True)

            # --- input DMAs, split across the two HWDGE queues ---
            # Transfers on one queue are processed in issue order at
            # ~150-230 GB/s, so each queue is an independent supply lane.
            # Scalar lane: x chunk 0 first (the start-gating transfer),
            # then even w_up j-tiles, then x chunks 1-2, then w_down tail.
            # Sync lane: w_up j0 (the other start-gating transfer), then
            # odd w_up j-tiles, then the first two w_down chunks.
            # Scalar lane: x chunk 0 first (the start-gating transfer),
            # then even w_up j-tiles, then x chunks 1-2, then the w_down
            # tail (needed only ~55us in, at GEMM2). Sync lane: w_up j0
            # (the other gating transfer), odd j-tiles, first two w_down.
            q_scalar = [("x", 0)] + [("wu", j) for j in range(2, n_j, 2)]
            q_scalar += [("x", 1), ("x", 2), ("wd", 2), ("wd", 3)]
            q_sync = [("wu", 0)] + [("wu", j) for j in range(1, n_j, 2)]
            q_sync += [("wd", 0), ("wd", 1)]

            def issue(eng, kind, i):
                if kind == "x":
                    s, l = C_CHUNKS[i]
                    eng.dma_start(
                        out=xT_sb[:, n_d * s:n_d * (s + l)], in_=xch[i][:]
                    )
                elif kind == "wu":
                    eng.dma_start(out=wuT_sb[:, i], in_=wuch[i][:])
                else:
                    eng.dma_start(
                        out=wdT_sb[:, i * 4:(i + 1) * 4, :], in_=wdch[i][:]
                    )

            for kind, i in q_scalar:
                issue(nc.scalar, kind, i)
            for kind, i in q_sync:
                issue(nc.sync, kind, i)

            # --- GEMM1 + relu^2: hsq[j, t] ---
            for ci, (s, l) in enumerate(C_CHUNKS):
                xbase = n_d * s
                for j in range(n_j):
                    ps = psum.tile([P, FD], f32, tag="ps")
                    for d in range(n_d):
                        nc.tensor.matmul(
                            ps[:, 0:l],
                            wuT_sb[:, j, d],
                            xT_sb[:, xbase + d * l:xbase + (d + 1) * l],
                            start=(d == 0),
                            stop=(d == n_d - 1),
                        )
                    hr = actp.tile([P, FD], bf16, tag="hr")
                    nc.vector.tensor_relu(out=hr[:, 0:l], in_=ps[:, 0:l])
                    nc.vector.tensor_mul(
                        out=hsq_sb[:, j, s:s + l], in0=hr[:, 0:l], in1=hr[:, 0:l]
                    )

            # --- GEMM2: y[t, i] = sum_j hsq[j, t].T @ wdT[j, i] ---
            groups = []
            for t in range(n_t):
                for ic in range(D // FD):
                    lo, hi = ic * FD, (ic + 1) * FD
                    if t == n_t - 1 and hi == D and LAST_SPLIT:
                        groups.append((t, lo, hi - LAST_SPLIT))
                        groups.append((t, hi - LAST_SPLIT, hi))
                    else:
                        groups.append((t, lo, hi))
            for gi, (t, lo, hi) in enumerate(groups):
                w = hi - lo
                ps = psum.tile([P, FD], f32, tag="ps")
                for j in range(n_j):
                    nc.tensor.matmul(
                        ps[:, 0:w],
                        hsq_sb[:, j, t * P:(t + 1) * P],
                        wdT_sb[:, j, lo:hi],
                        start=(j == 0),
                        stop=(j == n_j - 1),
                    )
                # The very last group drains serially after the final
                # matmul: cast + DMA it in two halves so the first DMA
                # trigger overlaps the second cast and the final transfer
                # is half the bytes.
                halves = [(0, w, nc.sync)] if gi < len(groups) - 1 else [
                    (0, w // 2, nc.sync), (w // 2, w, nc.scalar)
                ]
                for a, b, eng in halves:
                    yt = outp.tile([P, FD], bf16, tag="yt")
                    nc.vector.tensor_copy(out=yt[:, 0:b - a], in_=ps[:, a:b])
                    eng.dma_start(
                        out=y[:, t * D + lo + a:t * D + lo + b],
                        in_=yt[:, 0:b - a],
                    )

    nc.compile()
    return nc


def _get_program(cap: int) -> "bass.Bass":
    nc = _PROGRAM_CACHE.get(cap)
    if nc is None:
        nc = _build_program(cap)
        _PROGRAM_CACHE[cap] = nc
    return nc


CAP = 1024  # tokens per core per round (the uniform T/E split = one round)


def kernel(x, num_tokens_per_expert, w_up, w_down, _trace=False):
    global LAST_RESULT
    bf = ml_dtypes.bfloat16
    x = np.asarray(x)
    counts = np.asarray(num_tokens_per_expert).astype(np.int64)
    w_up = np.asarray(w_up)
    w_down = np.asarray(w_down)
    n_tok = x.shape[0]
    assert counts.shape == (E,) and int(counts.sum()) == n_tok
    offsets = np.zeros(E, dtype=np.int64)
    offsets[1:] = np.cumsum(counts)[:-1]

    nc = _get_program(CAP)
    n_d, n_j, n_t = D // P, H // P, CAP // P

    # Work list: split each expert's contiguous token segment into slots of
    # <= CAP tokens; process 8 slots per SPMD round. The uniform T/E = 1024
    # split is exactly one round of 8 slots.
    slots = []
    for e in range(E):
        cnt, off = int(counts[e]), int(offsets[e])
        for s in range(0, cnt, CAP):
            slots.append((e, off + s, min(CAP, cnt - s)))

    weight_cache = {}

    def expert_weights(e):
        if e not in weight_cache:
            wuT = np.ascontiguousarray(w_up[e].astype(bf).T)    # [D, H]
            wdT = np.ascontiguousarray(w_down[e].astype(bf).T)  # [H, D]
            wu3 = wuT.reshape(n_d, P, H)
            m = {
                f"wu{j}": np.ascontiguousarray(
                    wu3[:, :, j * P:(j + 1) * P].transpose(1, 0, 2)
                ).reshape(P, n_d * P)
                for j in range(n_j)
            }
            wd3 = wdT.reshape(n_j, P, D)
            for c in range(n_j // 4):
                m[f"wd{c}"] = np.ascontiguousarray(
                    wd3[c * 4:(c + 1) * 4].transpose(1, 0, 2)
                ).reshape(P, 4 * D)
            weight_cache[e] = m
        return weight_cache[e]

    out = np.zeros((n_tok, D), dtype=x.dtype)
    zero_map = None
    for r0 in range(0, len(slots), N_CORES):
        round_slots = slots[r0:r0 + N_CORES]
        in_maps = []
        for e, off, cnt in round_slots:
            xs = np.zeros((CAP, D), dtype=bf)
            xs[:cnt] = x[off:off + cnt].astype(bf)
            xT = np.ascontiguousarray(xs.T)  # [D, CAP]
            xT3 = xT.reshape(n_d, P, CAP)
            im = dict(expert_weights(e))
            for c, (s, l) in enumerate(C_CHUNKS):
                im[f"x{c}"] = np.ascontiguousarray(
                    xT3[:, :, s:s + l].transpose(1, 0, 2)
                ).reshape(P, n_d * l)
            in_maps.append(im)
        while len(in_maps) < N_CORES:  # idle cores in the last round
            if zero_map is None:
                zero_map = {
                    f"x{c}": np.zeros((P, n_d * l), dtype=bf)
                    for c, (_, l) in enumerate(C_CHUNKS)
                }
                zero_map.update({
                    f"wu{j}": np.zeros((P, n_d * P), dtype=bf)
                    for j in range(n_j)
                })
                zero_map.update({
                    f"wd{c}": np.zeros((P, 4 * D), dtype=bf)
                    for c in range(n_j // 4)
                })
            in_maps.append(zero_map)

        res = run_bass_kernel_spmd(
            nc, in_maps, core_ids=list(range(N_CORES)), trace=_trace
        )
        LAST_RESULT = res
        for i, (e, off, cnt) in enumerate(round_slots):
            yp = res.results[i]["y"].reshape(P, n_t, D).transpose(1, 0, 2)
            out[off:off + cnt] = yp.reshape(CAP, D)[:cnt].astype(x.dtype)
    return out
